# revision 21
# baseline (speedup 1.0000x reference)
"""NextVLAD Trainium2 kernel: 8-core SPMD bass/tile implementation.

Strategy:
  * Host folds W_emb@W1 (and the centroid/attention projections) so the
    front end contracts over K=144 instead of K=2048.
  * Front end is data-parallel over batch (16 images/core, rows padded
    to 64 per image).
  * The 16.8MB context-gating weight slice streams from HBM in a
    contiguous chunk layout through a 12-deep SBUF ring; the first 12
    chunks are issued at t=0 so they overlap the front end.
  * VLAD descriptors (bf16) are AllGathered; the big context-gating
    matmul is column-sharded (each core owns 256 of 2048 output cols,
    full batch). The gathered vlads are read back transposed via the
    XBAR transpose-DMA (no PE transposes), so BatchNorm reduces along
    the free axis.
  * One small AllReduce for the gating bottleneck contraction; final
    [128, 10] partials are summed on the host.
"""

import numpy as np
import ml_dtypes

import concourse.bass as bass
import concourse.mybir as mybir
import concourse.tile as tile
from concourse import bacc, bass_utils

F32 = mybir.dt.float32
F32R = mybir.dt.float32r
BF16 = mybir.dt.bfloat16
F16 = mybir.dt.float16

B, T, POSE = 128, 60, 144
DIM, EXP, GRP, K, NCLS = 2048, 2, 8, 64, 10
ED = EXP * DIM            # 4096
FS = ED // GRP            # 512
VLAD = K * FS             # 32768
HID = DIM                 # cg hidden
RED = HID // 8            # gating reduction

NCORES = 8
BPC = B // NCORES         # 16 images per core
TP = 64                   # padded rows per image (60 real + 4 pad)
ROWS = BPC * TP           # 1024 rows per core
RT = ROWS // 128          # 8 row tiles
KA = POSE + 1             # 145 contraction (with bias row)
COLS_C = HID // NCORES    # 256 y-columns per core
EPS = 1e-5

NCH = 32                  # wcg chunks (8 k-tiles each)
WBUFS = 12                # wcg chunk ring depth in SBUF
NKT = VLAD // 128         # 256 k-tiles in context gating

_CACHED = {}
TRACE = False          # test.py sets this to capture an NTFF profile
LAST_RESULT = None     # BassKernelResults from the most recent run


def _build_nc(reps=1, collectives=True):
    nc = bacc.Bacc("TRN2", target_bir_lowering=False, debug=False,
                   num_devices=NCORES)

    xT = nc.dram_tensor("xT", [KA, ROWS], F32R, kind="ExternalInput").ap()
    wh = nc.dram_tensor("wh", [KA, ED], F32R, kind="ExternalInput").ap()
    wl = nc.dram_tensor("wl", [KA, FS + GRP], F32R, kind="ExternalInput").ap()
    c2t = nc.dram_tensor("c2t", [K, FS], F32, kind="ExternalInput").ap()
    wcg = nc.dram_tensor("wcg", [NCH, 128, 8 * COLS_C], F16,
                         kind="ExternalInput").ap()
    bn1g = nc.dram_tensor("bn1g", [COLS_C], F32, kind="ExternalInput").ap()
    bn1b = nc.dram_tensor("bn1b", [COLS_C], F32, kind="ExternalInput").ap()
    wg1 = nc.dram_tensor("wg1", [COLS_C, RED], F32R, kind="ExternalInput").ap()
    bn2g = nc.dram_tensor("bn2g", [RED], F32, kind="ExternalInput").ap()
    bn2b = nc.dram_tensor("bn2b", [RED], F32, kind="ExternalInput").ap()
    wg2 = nc.dram_tensor("wg2", [RED, COLS_C], F32R, kind="ExternalInput").ap()
    bg2 = nc.dram_tensor("bg2", [COLS_C], F32, kind="ExternalInput").ap()
    w3 = nc.dram_tensor("w3", [COLS_C, NCLS], F32R, kind="ExternalInput").ap()
    rowmask = nc.dram_tensor("rowmask", [128, 1], F32, kind="ExternalInput").ap()
    ind2_in = nc.dram_tensor("ind2_in", [128, 2], F32R, kind="ExternalInput").ap()
    identr_in = nc.dram_tensor("identr_in", [128, 128], F32R, kind="ExternalInput").ap()
    ones_in = nc.dram_tensor("ones_in", [128, 1], F32R, kind="ExternalInput").ap()
    out_part = nc.dram_tensor("out_part", [B, NCLS], F32,
                              kind="ExternalOutput").ap()

    AF = mybir.ActivationFunctionType
    AX = mybir.AxisListType

    with tile.TileContext(nc) as tc:
      for _rep in range(reps):
          with tc.tile_pool(name="const", bufs=1) as const, \
               tc.tile_pool(name="cgw", bufs=1) as wpool, \
               tc.tile_pool(name="dram", bufs=1, space="DRAM") as dram:
              # ---- constants loaded once ----
              wh0 = const.tile([128, ED], F32R)
              wh1 = const.tile([KA - 128, ED], F32R)
              nc.sync.dma_start(wh0[:], wh[0:128, :])
              nc.sync.dma_start(wh1[:], wh[128:KA, :])
              wl0 = const.tile([128, FS + GRP], F32R)
              wl1 = const.tile([KA - 128, FS + GRP], F32R)
              nc.sync.dma_start(wl0[:], wl[0:128, :])
              nc.sync.dma_start(wl1[:], wl[128:KA, :])
              c2t_sb = const.tile([K, FS], F32)
              nc.sync.dma_start(c2t_sb[:], c2t[:])
              ind2 = const.tile([128, 2], F32R)
              nc.sync.dma_start(ind2[:], ind2_in[:])
              eps1 = const.tile([128, 1], F32)
              nc.any.memset(eps1[:, :], EPS)
              rmask_sb = const.tile([128, 1], F32)
              nc.sync.dma_start(rmask_sb[:], rowmask[:])

              # tail weights (small, resident)
              bn2g_sb, bn2b_sb, bg2_sb = [], [], []
              wg1_sb, wg2_sb, w3_sb = {}, {}, []
              bn1g_row = const.tile([1, COLS_C], F32)
              nc.sync.dma_start(bn1g_row[:],
                                bn1g[:].rearrange("(o c) -> o c", o=1))
              bn1b_row = const.tile([1, COLS_C], F32)
              nc.sync.dma_start(bn1b_row[:],
                                bn1b[:].rearrange("(o c) -> o c", o=1))
              identr = const.tile([128, 128], F32R)
              nc.sync.dma_start(identr[:], identr_in[:])
              ones_col = const.tile([128, 1], F32R)
              nc.sync.dma_start(ones_col[:], ones_in[:])
              ones_row = const.tile([1, 128], F32R)
              nc.sync.dma_start(ones_row[:],
                                ones_in[:].rearrange("p o -> o p"))
              for mi in range(2):
                  t = const.tile([128, 1], F32, tag=f"bn2g{mi}")
                  nc.sync.dma_start(t[:], bn2g[mi * 128:(mi + 1) * 128]
                                    .rearrange("(p o) -> p o", o=1))
                  bn2g_sb.append(t)
                  t = const.tile([128, 1], F32, tag=f"bn2b{mi}")
                  nc.sync.dma_start(t[:], bn2b[mi * 128:(mi + 1) * 128]
                                    .rearrange("(p o) -> p o", o=1))
                  bn2b_sb.append(t)
                  t = const.tile([128, 1], F32, tag=f"bg2{mi}")
                  nc.sync.dma_start(t[:], bg2[mi * 128:(mi + 1) * 128]
                                    .rearrange("(p o) -> p o", o=1))
                  bg2_sb.append(t)
                  t = const.tile([128, NCLS], F32R, tag=f"w3{mi}")
                  nc.sync.dma_start(t[:], w3[mi * 128:(mi + 1) * 128, :])
                  w3_sb.append(t)
                  for kt in range(2):
                      t = const.tile([128, 128], F32R, tag=f"wg1_{kt}{mi}")
                      nc.sync.dma_start(t[:], wg1[kt * 128:(kt + 1) * 128,
                                                  mi * 128:(mi + 1) * 128])
                      wg1_sb[(kt, mi)] = t
                      t = const.tile([128, 128], F32R, tag=f"wg2_{kt}{mi}")
                      nc.sync.dma_start(t[:], wg2[kt * 128:(kt + 1) * 128,
                                                  mi * 128:(mi + 1) * 128])
                      wg2_sb[(kt, mi)] = t

              vlads_own = dram.tile([BPC, VLAD], F16)
              vlads_all = dram.tile([B, VLAD], F16,
                                    addr_space="Shared" if collectives else "Local")

              # wcg chunk ring: first WBUFS chunks issued now (no waits
              # possible -> they stream during the front end); the rest are
              # issued at the top of the context-gating section.
              wch_tiles = {}

              def load_wch(ch):
                  t = wpool.tile([128, 8 * COLS_C], F16, tag="wch",
                                 name="wch", bufs=WBUFS)
                  nc.sync.dma_start(t[:], wcg[ch])
                  wch_tiles[ch] = t

              for ch in range(WBUFS):
                  load_wch(ch)

              # ================= front end =================
              with tc.tile_pool(name="fex", bufs=3) as xpool, \
                   tc.tile_pool(name="feh", bufs=2) as hpool, \
                   tc.tile_pool(name="fel", bufs=2) as lpool, \
                   tc.tile_pool(name="feaux", bufs=3) as aux, \
                   tc.tile_pool(name="fevo", bufs=3) as vopool, \
                   tc.tile_pool(name="ph", bufs=3, space="PSUM") as phpool, \
                   tc.tile_pool(name="pv", bufs=2, space="PSUM") as pvpool, \
                   tc.tile_pool(name="psml", bufs=2, space="PSUM") as psml:
                  for rt in range(RT):
                      rs = rt * 128
                      xk0 = xpool.tile([128, 128], F32R, tag="xk0")
                      xk1 = xpool.tile([KA - 128, 128], F32R, tag="xk1")
                      nc.sync.dma_start(xk0[:], xT[0:128, rs:rs + 128])
                      nc.sync.dma_start(xk1[:], xT[128:KA, rs:rs + 128])

                      h_sb = hpool.tile([128, ED], F32R, tag="h")
                      for nt in range(ED // 512):
                          ph = phpool.tile([128, 512], F32, tag="ph", bufs=2)
                          nc.tensor.matmul(ph[:], xk0[:],
                                           wh0[:, nt * 512:(nt + 1) * 512],
                                           start=True, stop=False)
                          nc.tensor.matmul(ph[:], xk1[:],
                                           wh1[:, nt * 512:(nt + 1) * 512],
                                           start=False, stop=True)
                          nc.scalar.copy(h_sb[:, nt * 512:(nt + 1) * 512], ph[:])

                      pl = phpool.tile([128, 512], F32, tag="pl", bufs=1)
                      nc.tensor.matmul(pl[:], xk0[:], wl0[:, 0:512],
                                       start=True, stop=False)
                      nc.tensor.matmul(pl[:], xk1[:], wl1[:, 0:512],
                                       start=False, stop=True)
                      L_sb = lpool.tile([128, 512], F32, tag="L")
                      nc.scalar.copy(L_sb[:], pl[:])

                      pa = psml.tile([128, GRP], F32, tag="pa", bufs=1)
                      nc.tensor.matmul(pa[:], xk0[:], wl0[:, 512:512 + GRP],
                                       start=True, stop=False)
                      nc.tensor.matmul(pa[:], xk1[:], wl1[:, 512:512 + GRP],
                                       start=False, stop=True)
                      att_sb = aux.tile([128, GRP], F32, tag="att")
                      nc.scalar.activation(att_sb[:], pa[:], AF.Sigmoid)

                      # softmax over each group of 64 columns
                      nmx = aux.tile([128, GRP], F32, tag="nmx")
                      for g in range(GRP):
                          nc.vector.reduce_max(nmx[:, g:g + 1],
                                               L_sb[:, g * 64:(g + 1) * 64],
                                               axis=AX.X, negate=True)
                      act_sb = lpool.tile([128, 512], F32R, tag="act")
                      sums = aux.tile([128, GRP], F32, tag="sums")
                      for g in range(GRP):
                          nc.scalar.activation(act_sb[:, g * 64:(g + 1) * 64],
                                               L_sb[:, g * 64:(g + 1) * 64],
                                               AF.Exp, bias=nmx[:, g:g + 1],
                                               accum_out=sums[:, g:g + 1])
                      rec = aux.tile([128, GRP], F32, tag="rec")
                      nc.vector.reciprocal(rec[:], sums[:])
                      scl = aux.tile([128, GRP], F32, tag="scl")
                      nc.vector.tensor_mul(scl[:], att_sb[:], rec[:])
                      nc.vector.tensor_scalar_mul(scl[:], scl[:], rmask_sb[:])
                      for g in range(GRP):
                          nc.vector.tensor_scalar_mul(
                              act_sb[:, g * 64:(g + 1) * 64],
                              act_sb[:, g * 64:(g + 1) * 64], scl[:, g:g + 1])

                      ls = aux.tile([128, 64], F32R, tag="ls")
                      nc.vector.tensor_add(ls[:], act_sb[:, 0:64],
                                           act_sb[:, 64:128])
                      for g in range(2, GRP):
                          nc.vector.tensor_add(ls[:], ls[:],
                                               act_sb[:, g * 64:(g + 1) * 64])
                      ps2 = psml.tile([64, 2], F32, tag="ps2", bufs=1)
                      nc.tensor.matmul(ps2[:], ls[:], ind2[:],
                                       start=True, stop=True)
                      s_sb = aux.tile([64, 2], F32, tag="s")
                      nc.scalar.copy(s_sb[:], ps2[:])

                      vout = vopool.tile([64, 2 * 512], F16, tag="vout")
                      for b2 in range(2):
                          r0 = b2 * 64
                          pv = pvpool.tile([64, 512], F32, tag="pv", bufs=2)
                          for g in range(GRP):
                              nc.tensor.matmul(
                                  pv[:],
                                  act_sb[r0:r0 + 64, g * 64:(g + 1) * 64],
                                  h_sb[r0:r0 + 64, g * 512:(g + 1) * 512],
                                  start=(g == 0), stop=(g == GRP - 1))
                          tmp = vopool.tile([64, 512], F32, tag="tmpc2")
                          nc.vector.tensor_scalar_mul(tmp[:], c2t_sb[:],
                                                      s_sb[:, b2:b2 + 1])
                          nc.vector.tensor_sub(
                              vout[:, b2 * 512:(b2 + 1) * 512], pv[:], tmp[:])
                      bg = rt * 2
                      nc.sync.dma_start(
                          vlads_own[bg:bg + 2].rearrange("b (k f) -> k b f", k=64),
                          vout[:].rearrange("k (b f) -> k b f", b=2))

              # ================= all-gather =================
              if collectives:
                  nc.gpsimd.collective_compute(
                      "AllGather", mybir.AluOpType.bypass,
                      replica_groups=[list(range(NCORES))],
                      ins=[vlads_own.opt()], outs=[vlads_all.opt()])
              else:
                  for _c in range(NCORES):
                      nc.sync.dma_start(
                          vlads_all[_c * BPC:(_c + 1) * BPC, :],
                          vlads_own[:, :])

              # ================= context gating =================
              with tc.tile_pool(name="cgv", bufs=2) as vpool, \
                   tc.tile_pool(name="cgsb", bufs=2) as cgsb, \
                   tc.tile_pool(name="cgaux", bufs=4) as cga, \
                   tc.tile_pool(name="cgp", bufs=1, space="PSUM") as cgps, \
                   tc.tile_pool(name="cgpt", bufs=2, space="PSUM") as cgpt, \
                   tc.tile_pool(name="cgp2", bufs=2, space="PSUM") as cgps2:
                  # stream the rest of the wcg ring
                  for ch in range(WBUFS, NCH):
                      load_wch(ch)

                  py = cgps.tile([128, COLS_C], F32, tag="py", bufs=1)
                  for vch in range(NKT // 16):
                      vt = vpool.tile([128, 16, 128], F16, tag="vt", bufs=2)
                      nc.sync.dma_start(
                          vt[:], vlads_all[:, vch * 2048:(vch + 1) * 2048],
                          transpose=True)
                      for j in range(16):
                          kt = vch * 16 + j
                          ch, k8 = kt // 8, kt % 8
                          nc.tensor.matmul(
                              py[:], vt[:, j, :],
                              wch_tiles[ch][:, k8 * COLS_C:(k8 + 1) * COLS_C],
                              start=(kt == 0), stop=(kt == NKT - 1))

                  # ---- BN1 over batch (batch on partitions -> PE stats) ----
                  y_sb = cgsb.tile([128, COLS_C], F32R, tag="y_sb")
                  nc.scalar.copy(y_sb[:], py[:])
                  sq = cgsb.tile([128, COLS_C], F32R, tag="sq")
                  nc.vector.tensor_mul(sq[:], y_sb[:], y_sb[:])
                  pstat = cgps2.tile([1, 2 * COLS_C], F32, tag="pstat_po", name="pstat", bufs=1)
                  nc.tensor.matmul(pstat[:, 0:COLS_C], ones_col[:], y_sb[:],
                                   start=True, stop=False)
                  nc.tensor.matmul(pstat[:, COLS_C:2 * COLS_C], ones_col[:],
                                   sq[:], start=True, stop=True,
                                   skip_group_check=True)
                  mu = cga.tile([1, COLS_C], F32, tag="mu")
                  nc.vector.tensor_scalar_mul(mu[:], pstat[:, 0:COLS_C],
                                              1.0 / B)
                  ex2 = cga.tile([1, COLS_C], F32, tag="ex2")
                  nc.vector.tensor_scalar_mul(ex2[:], pstat[:, COLS_C:],
                                              1.0 / B)
                  musq = cga.tile([1, COLS_C], F32, tag="musq")
                  nc.vector.tensor_mul(musq[:], mu[:], mu[:])
                  var = cga.tile([1, COLS_C], F32, tag="var")
                  nc.vector.tensor_sub(var[:], ex2[:], musq[:])
                  sd = cga.tile([1, COLS_C], F32, tag="sd")
                  nc.scalar.activation(sd[:], var[:], AF.Sqrt, bias=eps1[0:1, 0:1])
                  rstd = cga.tile([1, COLS_C], F32, tag="rstd")
                  nc.vector.reciprocal(rstd[:], sd[:])
                  seff = cga.tile([1, COLS_C], F32R, tag="seff")
                  nc.vector.tensor_mul(seff[:], bn1g_row[:], rstd[:])
                  mue = cga.tile([1, COLS_C], F32, tag="mue")
                  nc.vector.tensor_mul(mue[:], mu[:], seff[:])
                  beff = cga.tile([1, COLS_C], F32R, tag="beff")
                  nc.vector.tensor_sub(beff[:], bn1b_row[:], mue[:])
                  prep = cgps2.tile([128, 2 * COLS_C], F32, tag="prep", bufs=1)
                  nc.tensor.matmul(prep[:, 0:COLS_C], ones_row[:], seff[:],
                                   start=True, stop=False)
                  nc.tensor.matmul(prep[:, COLS_C:2 * COLS_C], ones_row[:], beff[:],
                                   start=True, stop=True, skip_group_check=True)
                  ybn = cgsb.tile([128, COLS_C], F32R, tag="ybn")
                  nc.vector.tensor_mul(ybn[:], y_sb[:], prep[:, 0:COLS_C])
                  nc.vector.tensor_add(ybn[:], ybn[:], prep[:, COLS_C:2 * COLS_C])

                  # transpose ybn -> [cols, b] tiles for the gating contraction
                  ybnT = []
                  for ct in range(2):
                      ptr = cgpt.tile([128, 128], F32R, tag="ptr", bufs=1)
                      nc.tensor.transpose(ptr[:], ybn[:, ct * 128:(ct + 1) * 128],
                                          identr[:])
                      yt = cgsb.tile([128, 128], F32R, tag=f"ybnT{ct}",
                                     name=f"ybnT{ct}")
                      nc.vector.tensor_copy(yt[:], ptr[:])
                      ybnT.append(yt)

                  # z partial, AllReduce, BN2+relu
                  z_in = dram.tile([RED, B], F32)
                  z_out = dram.tile([RED, B], F32,
                                    addr_space="Shared" if collectives else "Local")
                  for mt in range(2):
                      pz = cgps2.tile([128, B], F32, tag="pzg", name="pz", bufs=1)
                      for kt in range(2):
                          nc.tensor.matmul(pz[:], wg1_sb[(kt, mt)][:],
                                           ybnT[kt][:],
                                           start=(kt == 0), stop=(kt == 1))
                      zp = cgsb.tile([128, B], F32, tag="zp")
                      nc.scalar.copy(zp[:], pz[:])
                      nc.sync.dma_start(z_in[mt * 128:(mt + 1) * 128, :], zp[:])

                  if collectives:
                      nc.gpsimd.collective_compute(
                          "AllReduce", mybir.AluOpType.add,
                          replica_groups=[list(range(NCORES))],
                          ins=[z_in.opt()], outs=[z_out.opt()])
                  else:
                      nc.sync.dma_start(z_out[:, :], z_in[:, :])

                  rT = []
                  for mt in range(2):
                      zt = cgsb.tile([128, B], F32, tag=f"zt{mt}", name=f"zt{mt}")
                      nc.sync.dma_start(zt[:], z_out[mt * 128:(mt + 1) * 128, :])
                      rs_ = cga.tile([128, 1], F32, tag="rs")
                      nc.vector.reduce_sum(rs_[:], zt[:], axis=AX.X)
                      muz = cga.tile([128, 1], F32, tag="muz")
                      nc.vector.tensor_scalar_mul(muz[:], rs_[:], 1.0 / B)
                      xc = cgsb.tile([128, B], F32, tag="xcz")
                      nc.vector.tensor_scalar_sub(xc[:], zt[:], muz[:])
                      sqz = cgsb.tile([128, B], F32, tag="sqz")
                      nc.vector.tensor_mul(sqz[:], xc[:], xc[:])
                      vs = cga.tile([128, 1], F32, tag="vs")
                      nc.vector.reduce_sum(vs[:], sqz[:], axis=AX.X)
                      varz = cga.tile([128, 1], F32, tag="varz")
                      nc.vector.tensor_scalar_mul(varz[:], vs[:], 1.0 / B)
                      sdz = cga.tile([128, 1], F32, tag="sdz")
                      nc.scalar.activation(sdz[:], varz[:], AF.Sqrt, bias=eps1[:, 0:1])
                      rstdz = cga.tile([128, 1], F32, tag="rstdz")
                      nc.vector.reciprocal(rstdz[:], sdz[:])
                      seffz = cga.tile([128, 1], F32, tag="seffz")
                      nc.vector.tensor_mul(seffz[:], bn2g_sb[mt][:], rstdz[:])
                      tmpz = cgsb.tile([128, B], F32, tag="tmpz")
                      nc.vector.tensor_scalar_mul(tmpz[:], xc[:], seffz[:])
                      nc.vector.tensor_scalar_add(tmpz[:], tmpz[:], bn2b_sb[mt][:])
                      rt_ = cgsb.tile([128, B], F32R, tag=f"rT{mt}", name=f"rT{mt}")
                      nc.scalar.activation(rt_[:], tmpz[:], AF.Relu)
                      rT.append(rt_)

                  # gate and output
                  po = cgps2.tile([128, NCLS], F32, tag="pstat_po", name="po", bufs=1)
                  for mt in range(2):
                      pg = cgps2.tile([128, B], F32, tag="pzg", name="pg", bufs=1)
                      for kt in range(2):
                          nc.tensor.matmul(pg[:], wg2_sb[(kt, mt)][:],
                                           rT[kt][:],
                                           start=(kt == 0), stop=(kt == 1))
                      gate = cgsb.tile([128, B], F32R, tag="gate")
                      nc.scalar.activation(gate[:], pg[:], AF.Sigmoid,
                                           bias=bg2_sb[mt][:])
                      o_sb = cgsb.tile([128, B], F32R, tag="o")
                      nc.vector.tensor_mul(o_sb[:], ybnT[mt][:], gate[:])
                      nc.tensor.matmul(po[:], o_sb[:], w3_sb[mt][:],
                                       start=(mt == 0), stop=(mt == 1))
                  out_sb = cgsb.tile([128, NCLS], F32, tag="outp")
                  nc.scalar.copy(out_sb[:], po[:])
                  nc.sync.dma_start(out_part[:], out_sb[:])

    nc.compile()
    return nc


def _host_prep(inputs):
    f32 = np.float32
    g = {k: np.asarray(v, dtype=f32) for k, v in inputs.items()}

    x2 = np.transpose(g["x"], (0, 3, 1, 2)).reshape(B, T, POSE)
    Wh = g["W_emb"] @ g["W1"]                       # [144, 4096]
    bh = g["b_emb"] @ g["W1"] + g["b1"]             # [4096]
    C1cat = np.concatenate([g["centroids1"], g["W2"]], axis=1)  # [4096, 520]
    WL = Wh @ C1cat                                 # [144, 520]
    bL = bh @ C1cat
    bL[FS:] += g["b2"]
    wh_aug = np.concatenate([Wh, bh[None, :]], axis=0).astype(f32)
    wl_aug = np.concatenate([WL, bL[None, :]], axis=0).astype(f32)
    c2t = np.ascontiguousarray(g["centroids2"][0].T)            # [64, 512]

    # permute Wcg rows: our flat vlad index is k*FS+f, reference is f*K+k
    new = np.arange(VLAD)
    old = (new % FS) * K + (new // FS)
    wcg_perm = g["Wcg"][old, :]                     # [32768, 2048]

    rmask = np.zeros((128, 1), f32)
    rmask[:T] = 1.0
    rmask[TP:TP + T] = 1.0
    ind2_np = np.zeros((128, 2), f32)
    ind2_np[0:TP, 0] = 1.0
    ind2_np[TP:128, 1] = 1.0
    identr_np = np.eye(128, dtype=f32)
    ones_np = np.ones((128, 1), f32)
    in_maps = []
    for c in range(NCORES):
        xs = x2[c * BPC:(c + 1) * BPC]              # [16, 60, 144]
        xp = np.zeros((BPC, TP, POSE), f32)
        xp[:, :T] = xs
        xT = np.ones((KA, ROWS), f32)
        xT[:POSE] = xp.reshape(ROWS, POSE).T
        sl = slice(c * COLS_C, (c + 1) * COLS_C)
        wsl = wcg_perm[:, sl].astype(np.float16)    # [32768, 256]
        wch = np.ascontiguousarray(
            wsl.reshape(NCH, 8, 128, COLS_C).transpose(0, 2, 1, 3)
            .reshape(NCH, 128, 8 * COLS_C))
        in_maps.append({
            "xT": np.ascontiguousarray(xT),
            "wh": wh_aug,
            "wl": wl_aug,
            "c2t": c2t,
            "wcg": wch,
            "bn1g": np.ascontiguousarray(g["g_bn1"][sl]),
            "bn1b": np.ascontiguousarray(g["b_bn1"][sl]),
            "wg1": np.ascontiguousarray(g["Wg1"][sl, :]),
            "bn2g": g["g_bn2"],
            "bn2b": g["b_bn2"],
            "wg2": np.ascontiguousarray(g["Wg2"][:, sl]),
            "bg2": np.ascontiguousarray(g["bg2"][sl]),
            "w3": np.ascontiguousarray(g["W3"][sl, :]),
            "rowmask": rmask,
            "ind2_in": ind2_np,
            "identr_in": identr_np,
            "ones_in": ones_np,
        })
    return in_maps, g["b3"]


def kernel(**inputs):
    if "nc" not in _CACHED:
        _CACHED["nc"] = _build_nc()
    nc = _CACHED["nc"]
    in_maps, b3 = _host_prep(inputs)
    global LAST_RESULT
    res = bass_utils.run_bass_kernel_spmd(nc, in_maps,
                                          core_ids=list(range(NCORES)),
                                          trace=TRACE)
    LAST_RESULT = res
    out = np.zeros((B, NCLS), np.float32)
    for c in range(NCORES):
        out += res.results[c]["out_part"]
    return out + b3[None, :]
